# revision 3
# baseline (speedup 1.0000x reference)
"""CrossAttention Trainium2 kernel.

Data-parallel over batch across 8 NeuronCores (4 batches each).
Host-side prep casts to bf16 and pre-transposes kv/q/weights so every
on-device matmul has its contraction dim on partitions; softmax skips
max-subtraction (logits are bounded ~|6|) and folds the additive mask in
multiplicatively via a host-precomputed exp(mask).
"""
import sys

sys.path.insert(0, "/opt/trn_rl_repo")

import numpy as np
import ml_dtypes

import concourse.bacc as bacc
import concourse.mybir as mybir
import concourse.tile as tile
from concourse.bass_utils import run_bass_kernel_spmd

BF = ml_dtypes.bfloat16

B, QN, N, DIM, HEADS, HD = 32, 128, 4096, 512, 8, 64
SCALE = HD ** -0.5
NCORES = 8
BL = B // NCORES  # batches per core
NT = N // 128     # 32 token tiles
NCH = 4           # n-chunks per head for QK/exp (1024 wide)
CHW = N // NCH    # 1024

f32 = mybir.dt.float32
bf16 = mybir.dt.bfloat16
MULT = mybir.AluOpType.mult
EXP = mybir.ActivationFunctionType.Exp

_built = None


def _emit(nc):
    kvT_d = nc.dram_tensor("kvT", [BL, 4, 128, N], bf16, kind="ExternalInput").ap()
    qT_d = nc.dram_tensor("qT", [4, 128, BL * QN], bf16, kind="ExternalInput").ap()
    em_d = nc.dram_tensor("em", [BL, QN, N], bf16, kind="ExternalInput").ap()
    wkvT_d = nc.dram_tensor("wkvT", [4, 128, 2 * DIM], bf16, kind="ExternalInput").ap()
    wqT_d = nc.dram_tensor("wqT", [4, 128, DIM], bf16, kind="ExternalInput").ap()
    wpT_d = nc.dram_tensor("wpT", [4, 128, DIM], bf16, kind="ExternalInput").ap()
    bias_d = nc.dram_tensor("biasb", [128, DIM], f32, kind="ExternalInput").ap()
    out_d = nc.dram_tensor("out", [BL, QN, DIM], f32, kind="ExternalOutput").ap()

    with tile.TileContext(nc) as tc:
        with (
            tc.tile_pool(name="wpool", bufs=1) as wpool,
            tc.tile_pool(name="kvtp", bufs=4) as kvtp,
            tc.tile_pool(name="ktp", bufs=4) as ktp,
            tc.tile_pool(name="vp", bufs=44) as vp,
            tc.tile_pool(name="pp", bufs=2) as pp,
            tc.tile_pool(name="ptp", bufs=2) as ptp,
            tc.tile_pool(name="emp", bufs=2) as emp,
            tc.tile_pool(name="xp", bufs=8) as xp,
            tc.tile_pool(name="outp", bufs=2) as outp,
            tc.tile_pool(name="smallp", bufs=8) as smallp,
            tc.tile_pool(name="mm512", bufs=2, space="PSUM") as mm512,
            tc.tile_pool(name="qkps", bufs=1, space="PSUM") as qkps,
            tc.tile_pool(name="avps", bufs=2, space="PSUM") as avps,
        ):
            # ---- persistent weights ----
            wkvT = []
            wqT = []
            wpT = []
            qT = []
            for t in range(4):
                wk = wpool.tile([128, 2 * DIM], bf16, name=f"wkvT{t}")
                nc.sync.dma_start(out=wk, in_=wkvT_d[t])
                wkvT.append(wk)
                wq = wpool.tile([128, DIM], bf16, name=f"wqT{t}")
                nc.sync.dma_start(out=wq, in_=wqT_d[t])
                wqT.append(wq)
                wp = wpool.tile([128, DIM], bf16, name=f"wpT{t}")
                nc.sync.dma_start(out=wp, in_=wpT_d[t])
                wpT.append(wp)
                qt = wpool.tile([128, BL * QN], bf16, name=f"qT{t}")
                nc.sync.dma_start(out=qt, in_=qT_d[t])
                qT.append(qt)
            bias_sb = wpool.tile([128, DIM], f32, name="bias_sb")
            nc.sync.dma_start(out=bias_sb, in_=bias_d)

            # ---- q projection for all local batches: qhT[co] = [c_out 128, (b q) 512]
            qhT = []
            for co in range(4):
                ps_q = mm512.tile([128, BL * QN], f32, name="ps_mm512")
                for ci in range(4):
                    nc.tensor.matmul(
                        ps_q,
                        wqT[ci][:, co * 128:(co + 1) * 128],
                        qT[ci],
                        start=(ci == 0),
                        stop=(ci == 3),
                    )
                qh = wpool.tile([128, BL * QN], bf16, name=f"qhT{co}")
                nc.any.tensor_copy(qh, ps_q)
                qhT.append(qh)

            for b in range(BL):
                # ---- load kvT (features x tokens) ----
                kvt = []
                for t in range(4):
                    kv_t = kvtp.tile([128, N], bf16, name="kv_t")
                    nc.gpsimd.dma_start(out=kv_t, in_=kvT_d[b, t])
                    kvt.append(kv_t)
                em_t = emp.tile([128, N], bf16, name="em_t")
                nc.gpsimd.dma_start(out=em_t, in_=em_d[b])

                # ---- k projection, feature-major: kt[ko] = [k_out 128, n 4096]
                kt = []
                for ko in range(4):
                    k_t = ktp.tile([128, N], bf16, name="k_t")
                    for ch in range(8):
                        ps_k = mm512.tile([128, 512], f32, name="ps_mm512")
                        for ci in range(4):
                            nc.tensor.matmul(
                                ps_k,
                                wkvT[ci][:, ko * 128:(ko + 1) * 128],
                                kvt[ci][:, ch * 512:(ch + 1) * 512],
                                start=(ci == 0),
                                stop=(ci == 3),
                            )
                        nc.any.tensor_copy(k_t[:, ch * 512:(ch + 1) * 512], ps_k)
                    kt.append(k_t)

                # ---- v projection, token-major: v[tt] = [n 128, v_feat 512]
                v = []
                for tt in range(NT):
                    ps_v = mm512.tile([128, 512], f32, name="ps_mm512")
                    for ci in range(4):
                        nc.tensor.matmul(
                            ps_v,
                            kvt[ci][:, tt * 128:(tt + 1) * 128],
                            wkvT[ci][:, DIM:2 * DIM],
                            start=(ci == 0),
                            stop=(ci == 3),
                        )
                    v_t = vp.tile([128, 512], bf16, name="v_t")
                    nc.any.tensor_copy(v_t, ps_v)
                    v.append(v_t)

                # ---- attention per head pair ----
                xT = []
                for pr in range(4):
                    # QK for both heads (row-packed K=64) + exp per chunk
                    p_sb = []
                    for hh in range(2):
                        p_h = pp.tile([128, N], bf16, name="p_h")
                        p_sb.append(p_h)
                    for ch in range(NCH):
                        ps_s0 = qkps.tile([128, CHW], f32, name="ps_s0")
                        ps_s1 = qkps.tile([128, CHW], f32, name="ps_s1")
                        for half in range(CHW // 512):
                            n0 = ch * CHW + half * 512
                            nc.tensor.matmul(
                                ps_s0[:, half * 512:(half + 1) * 512],
                                qhT[pr][0:64, b * QN:(b + 1) * QN],
                                kt[pr][0:64, n0:n0 + 512],
                                start=True,
                                stop=True,
                                tile_position=(0, 0),
                            )
                            nc.tensor.matmul(
                                ps_s1[:, half * 512:(half + 1) * 512],
                                qhT[pr][64:128, b * QN:(b + 1) * QN],
                                kt[pr][64:128, n0:n0 + 512],
                                start=True,
                                stop=True,
                                tile_position=(64, 0),
                            )
                        nc.scalar.activation(
                            p_sb[0][:, ch * CHW:(ch + 1) * CHW], ps_s0, EXP
                        )
                        nc.scalar.activation(
                            p_sb[1][:, ch * CHW:(ch + 1) * CHW], ps_s1, EXP
                        )

                    # mask-multiply + rowsum + normalize + transpose per head
                    pt_sb = []
                    for hh in range(2):
                        rowsum = smallp.tile([128, 1], f32, name="rowsum")
                        nc.vector.scalar_tensor_tensor(
                            out=p_sb[hh],
                            in0=p_sb[hh],
                            scalar=1.0,
                            in1=em_t,
                            op0=MULT,
                            op1=MULT,
                            accum_out=rowsum,
                        )
                        recip = smallp.tile([128, 1], f32, name="recip")
                        nc.vector.reciprocal(recip, rowsum)
                        nc.vector.tensor_scalar_mul(p_sb[hh], p_sb[hh], recip)
                        pt_h = ptp.tile([128, NT, 128], bf16, name="pt_h")
                        nc.sync.dma_start_transpose(pt_h, p_sb[hh])
                        pt_sb.append(pt_h)

                    # AV, column-tiled across the 2 heads
                    ps_x = avps.tile([128, QN], f32, name="ps_x")
                    for i in range(NT):
                        nc.tensor.matmul(
                            ps_x[0:64, :],
                            v[i][:, (2 * pr) * 64:(2 * pr + 1) * 64],
                            pt_sb[0][:, i, :],
                            start=(i == 0),
                            stop=(i == NT - 1),
                            tile_position=(0, 0),
                            skip_group_check=True,
                        )
                        nc.tensor.matmul(
                            ps_x[64:128, :],
                            v[i][:, (2 * pr + 1) * 64:(2 * pr + 2) * 64],
                            pt_sb[1][:, i, :],
                            start=(i == 0),
                            stop=(i == NT - 1),
                            tile_position=(0, 64),
                            skip_group_check=True,
                        )
                    x_t = xp.tile([128, QN], bf16, name="x_t")
                    nc.any.tensor_copy(x_t, ps_x)
                    xT.append(x_t)

                # ---- output projection: out[q, o] = sum_c xT[c,q]^T W^T[c,o]
                ps_o = mm512.tile([128, DIM], f32, name="ps_mm512")
                for pr in range(4):
                    nc.tensor.matmul(
                        ps_o, xT[pr], wpT[pr], start=(pr == 0), stop=(pr == 3)
                    )
                out_sb = outp.tile([128, DIM], f32, name="out_sb")
                nc.vector.tensor_add(out_sb, ps_o, bias_sb)
                nc.gpsimd.dma_start(out=out_d[b], in_=out_sb)
    return nc


def build():
    global _built
    if _built is None:
        nc = bacc.Bacc(
            "TRN2", target_bir_lowering=False, debug=False, num_devices=NCORES
        )
        _emit(nc)
        nc.compile()
        _built = nc
    return _built


def prep_inputs(q, kv, key_mask, Wq, Wkv, Wproj, bproj):
    """Host-side shard + layout prep. Returns per-core in_maps."""
    q = np.asarray(q, dtype=np.float32)
    kv = np.asarray(kv, dtype=np.float32)
    key_mask = np.asarray(key_mask, dtype=np.float32)
    wkvT = np.ascontiguousarray(np.asarray(Wkv, np.float32).T).astype(BF)
    wkvT = wkvT.reshape(4, 128, 2 * DIM)
    wqT = np.ascontiguousarray((np.asarray(Wq, np.float32) * SCALE).T).astype(BF)
    wqT = wqT.reshape(4, 128, DIM)
    wpT = np.ascontiguousarray(np.asarray(Wproj, np.float32).T).astype(BF)
    wpT = wpT.reshape(4, 128, DIM)
    biasb = np.ascontiguousarray(
        np.broadcast_to(np.asarray(bproj, np.float32), (128, DIM))
    )

    kv_bf = kv.astype(BF)
    em = np.exp(key_mask).astype(BF)

    in_maps = []
    for c in range(NCORES):
        sl = slice(c * BL, (c + 1) * BL)
        kvT = np.ascontiguousarray(kv_bf[sl].transpose(0, 2, 1)).reshape(
            BL, 4, 128, N
        )
        q_loc = q[sl].astype(BF)  # [BL, QN, DIM]
        qT = np.ascontiguousarray(q_loc.transpose(2, 0, 1)).reshape(4, 128, BL * QN)
        in_maps.append(
            {
                "kvT": kvT,
                "qT": qT,
                "em": np.ascontiguousarray(em[sl]),
                "wkvT": wkvT,
                "wqT": wqT,
                "wpT": wpT,
                "biasb": biasb,
            }
        )
    return in_maps


def kernel(q, kv, key_mask, Wq, Wkv, Wproj, bproj):
    nc = build()
    in_maps = prep_inputs(q, kv, key_mask, Wq, Wkv, Wproj, bproj)
    res = run_bass_kernel_spmd(nc, in_maps, core_ids=list(range(NCORES)))
    out = np.concatenate([res.results[c]["out"] for c in range(NCORES)], axis=0)
    return out.astype(np.float32)


# revision 5
# speedup vs baseline: 26.1410x; 26.1410x over previous
"""CrossAttention Trainium2 kernel.

Data-parallel over batch across 8 NeuronCores (4 batches each).
Host-side prep casts to bf16 and pre-transposes kv/q/weights so every
on-device matmul has its contraction dim on partitions; softmax skips
max-subtraction (logits are bounded ~|6|) and folds the additive mask in
multiplicatively via a host-precomputed exp(mask).
"""
import sys

sys.path.insert(0, "/opt/trn_rl_repo")

import numpy as np
import ml_dtypes

import concourse.bacc as bacc
import concourse.mybir as mybir
import concourse.tile as tile

BF = ml_dtypes.bfloat16

B, QN, N, DIM, HEADS, HD = 32, 128, 4096, 512, 8, 64
SCALE = HD ** -0.5
NCORES = 8
BL = B // NCORES  # batches per core
NT = N // 128     # 32 token tiles
NCH = 4           # n-chunks per head for QK/exp (1024 wide)
CHW = N // NCH    # 1024

f32 = mybir.dt.float32
bf16 = mybir.dt.bfloat16
MULT = mybir.AluOpType.mult
EXP = mybir.ActivationFunctionType.Exp

_built = None
_runner = None


def _emit(nc):
    kvT_d = nc.dram_tensor("kvT", [BL, 4, 128, N], bf16, kind="ExternalInput").ap()
    qT_d = nc.dram_tensor("qT", [4, 128, BL * QN], bf16, kind="ExternalInput").ap()
    em_d = nc.dram_tensor("em", [BL, QN, N], bf16, kind="ExternalInput").ap()
    wkvT_d = nc.dram_tensor("wkvT", [4, 128, 2 * DIM], bf16, kind="ExternalInput").ap()
    wqT_d = nc.dram_tensor("wqT", [4, 128, DIM], bf16, kind="ExternalInput").ap()
    wpT_d = nc.dram_tensor("wpT", [4, 128, DIM], bf16, kind="ExternalInput").ap()
    bias_d = nc.dram_tensor("biasb", [128, DIM], f32, kind="ExternalInput").ap()
    out_d = nc.dram_tensor("out", [BL, QN, DIM], f32, kind="ExternalOutput").ap()

    with tile.TileContext(nc) as tc:
        with (
            tc.tile_pool(name="wpool", bufs=1) as wpool,
            tc.tile_pool(name="kvtp", bufs=4) as kvtp,
            tc.tile_pool(name="ktp", bufs=4) as ktp,
            tc.tile_pool(name="vp", bufs=44) as vp,
            tc.tile_pool(name="pp", bufs=2) as pp,
            tc.tile_pool(name="ptp", bufs=2) as ptp,
            tc.tile_pool(name="emp", bufs=2) as emp,
            tc.tile_pool(name="xp", bufs=8) as xp,
            tc.tile_pool(name="outp", bufs=2) as outp,
            tc.tile_pool(name="smallp", bufs=8) as smallp,
            tc.tile_pool(name="mm512", bufs=2, space="PSUM") as mm512,
            tc.tile_pool(name="qkps", bufs=1, space="PSUM") as qkps,
            tc.tile_pool(name="avps", bufs=2, space="PSUM") as avps,
        ):
            # ---- persistent weights ----
            wkvT = []
            wqT = []
            wpT = []
            qT = []
            for t in range(4):
                wk = wpool.tile([128, 2 * DIM], bf16, name=f"wkvT{t}")
                nc.sync.dma_start(out=wk, in_=wkvT_d[t])
                wkvT.append(wk)
                wq = wpool.tile([128, DIM], bf16, name=f"wqT{t}")
                nc.sync.dma_start(out=wq, in_=wqT_d[t])
                wqT.append(wq)
                wp = wpool.tile([128, DIM], bf16, name=f"wpT{t}")
                nc.sync.dma_start(out=wp, in_=wpT_d[t])
                wpT.append(wp)
                qt = wpool.tile([128, BL * QN], bf16, name=f"qT{t}")
                nc.sync.dma_start(out=qt, in_=qT_d[t])
                qT.append(qt)
            bias_sb = wpool.tile([128, DIM], f32, name="bias_sb")
            nc.sync.dma_start(out=bias_sb, in_=bias_d)

            # ---- q projection for all local batches: qhT[co] = [c_out 128, (b q) 512]
            qhT = []
            for co in range(4):
                ps_q = mm512.tile([128, BL * QN], f32, name="ps_mm512")
                for ci in range(4):
                    nc.tensor.matmul(
                        ps_q,
                        wqT[ci][:, co * 128:(co + 1) * 128],
                        qT[ci],
                        start=(ci == 0),
                        stop=(ci == 3),
                    )
                qh = wpool.tile([128, BL * QN], bf16, name=f"qhT{co}")
                nc.any.tensor_copy(qh, ps_q)
                qhT.append(qh)

            for b in range(BL):
                # ---- load kvT (features x tokens) ----
                kvt = []
                for t in range(4):
                    kv_t = kvtp.tile([128, N], bf16, name="kv_t")
                    nc.gpsimd.dma_start(out=kv_t, in_=kvT_d[b, t])
                    kvt.append(kv_t)
                em_t = emp.tile([128, N], bf16, name="em_t")
                nc.gpsimd.dma_start(out=em_t, in_=em_d[b])

                # ---- k projection, feature-major: kt[ko] = [k_out 128, n 4096]
                kt = []
                for ko in range(4):
                    k_t = ktp.tile([128, N], bf16, name="k_t")
                    for ch in range(8):
                        ps_k = mm512.tile([128, 512], f32, name="ps_mm512")
                        for ci in range(4):
                            nc.tensor.matmul(
                                ps_k,
                                wkvT[ci][:, ko * 128:(ko + 1) * 128],
                                kvt[ci][:, ch * 512:(ch + 1) * 512],
                                start=(ci == 0),
                                stop=(ci == 3),
                            )
                        nc.any.tensor_copy(k_t[:, ch * 512:(ch + 1) * 512], ps_k)
                    kt.append(k_t)

                # ---- v projection, token-major: v[tt] = [n 128, v_feat 512]
                v = []
                for tt in range(NT):
                    ps_v = mm512.tile([128, 512], f32, name="ps_mm512")
                    for ci in range(4):
                        nc.tensor.matmul(
                            ps_v,
                            kvt[ci][:, tt * 128:(tt + 1) * 128],
                            wkvT[ci][:, DIM:2 * DIM],
                            start=(ci == 0),
                            stop=(ci == 3),
                        )
                    v_t = vp.tile([128, 512], bf16, name="v_t")
                    nc.any.tensor_copy(v_t, ps_v)
                    v.append(v_t)

                # ---- attention per head pair ----
                xT = []
                for pr in range(4):
                    # QK for both heads (row-packed K=64) + exp per chunk
                    p_sb = []
                    for hh in range(2):
                        p_h = pp.tile([128, N], bf16, name="p_h")
                        p_sb.append(p_h)
                    for ch in range(NCH):
                        ps_s0 = qkps.tile([128, CHW], f32, name="ps_s0")
                        ps_s1 = qkps.tile([128, CHW], f32, name="ps_s1")
                        for half in range(CHW // 512):
                            n0 = ch * CHW + half * 512
                            nc.tensor.matmul(
                                ps_s0[:, half * 512:(half + 1) * 512],
                                qhT[pr][0:64, b * QN:(b + 1) * QN],
                                kt[pr][0:64, n0:n0 + 512],
                                start=True,
                                stop=True,
                                tile_position=(0, 0),
                            )
                            nc.tensor.matmul(
                                ps_s1[:, half * 512:(half + 1) * 512],
                                qhT[pr][64:128, b * QN:(b + 1) * QN],
                                kt[pr][64:128, n0:n0 + 512],
                                start=True,
                                stop=True,
                                tile_position=(64, 0),
                            )
                        nc.scalar.activation(
                            p_sb[0][:, ch * CHW:(ch + 1) * CHW], ps_s0, EXP
                        )
                        nc.scalar.activation(
                            p_sb[1][:, ch * CHW:(ch + 1) * CHW], ps_s1, EXP
                        )

                    # mask-multiply + rowsum + normalize + transpose per head
                    pt_sb = []
                    for hh in range(2):
                        rowsum = smallp.tile([128, 1], f32, name="rowsum")
                        nc.vector.scalar_tensor_tensor(
                            out=p_sb[hh],
                            in0=p_sb[hh],
                            scalar=1.0,
                            in1=em_t,
                            op0=MULT,
                            op1=MULT,
                            accum_out=rowsum,
                        )
                        recip = smallp.tile([128, 1], f32, name="recip")
                        nc.vector.reciprocal(recip, rowsum)
                        nc.vector.tensor_scalar_mul(p_sb[hh], p_sb[hh], recip)
                        pt_h = ptp.tile([128, NT, 128], bf16, name="pt_h")
                        nc.sync.dma_start_transpose(pt_h, p_sb[hh])
                        pt_sb.append(pt_h)

                    # AV, column-tiled across the 2 heads
                    ps_x = avps.tile([128, QN], f32, name="ps_x")
                    for i in range(NT):
                        nc.tensor.matmul(
                            ps_x[0:64, :],
                            v[i][:, (2 * pr) * 64:(2 * pr + 1) * 64],
                            pt_sb[0][:, i, :],
                            start=(i == 0),
                            stop=(i == NT - 1),
                            tile_position=(0, 0),
                            skip_group_check=True,
                        )
                        nc.tensor.matmul(
                            ps_x[64:128, :],
                            v[i][:, (2 * pr + 1) * 64:(2 * pr + 2) * 64],
                            pt_sb[1][:, i, :],
                            start=(i == 0),
                            stop=(i == NT - 1),
                            tile_position=(0, 64),
                            skip_group_check=True,
                        )
                    x_t = xp.tile([128, QN], bf16, name="x_t")
                    nc.any.tensor_copy(x_t, ps_x)
                    xT.append(x_t)

                # ---- output projection: out[q, o] = sum_c xT[c,q]^T W^T[c,o]
                ps_o = mm512.tile([128, DIM], f32, name="ps_mm512")
                for pr in range(4):
                    nc.tensor.matmul(
                        ps_o, xT[pr], wpT[pr], start=(pr == 0), stop=(pr == 3)
                    )
                out_sb = outp.tile([128, DIM], f32, name="out_sb")
                nc.vector.tensor_add(out_sb, ps_o, bias_sb)
                nc.gpsimd.dma_start(out=out_d[b], in_=out_sb)
    return nc


def build():
    global _built
    if _built is None:
        nc = bacc.Bacc(
            "TRN2", target_bir_lowering=False, debug=False, num_devices=NCORES
        )
        _emit(nc)
        nc.compile()
        _built = nc
    return _built


def prep_inputs(q, kv, key_mask, Wq, Wkv, Wproj, bproj):
    """Host-side shard + layout prep. Returns per-core in_maps."""
    q = np.asarray(q, dtype=np.float32)
    kv = np.asarray(kv, dtype=np.float32)
    key_mask = np.asarray(key_mask, dtype=np.float32)
    wkvT = np.ascontiguousarray(np.asarray(Wkv, np.float32).T).astype(BF)
    wkvT = wkvT.reshape(4, 128, 2 * DIM)
    wqT = np.ascontiguousarray((np.asarray(Wq, np.float32) * SCALE).T).astype(BF)
    wqT = wqT.reshape(4, 128, DIM)
    wpT = np.ascontiguousarray(np.asarray(Wproj, np.float32).T).astype(BF)
    wpT = wpT.reshape(4, 128, DIM)
    biasb = np.ascontiguousarray(
        np.broadcast_to(np.asarray(bproj, np.float32), (128, DIM))
    )

    kv_bf = kv.astype(BF)
    em = np.exp(key_mask).astype(BF)

    in_maps = []
    for c in range(NCORES):
        sl = slice(c * BL, (c + 1) * BL)
        kvT = np.ascontiguousarray(kv_bf[sl].transpose(0, 2, 1)).reshape(
            BL, 4, 128, N
        )
        q_loc = q[sl].astype(BF)  # [BL, QN, DIM]
        qT = np.ascontiguousarray(q_loc.transpose(2, 0, 1)).reshape(4, 128, BL * QN)
        in_maps.append(
            {
                "kvT": kvT,
                "qT": qT,
                "em": np.ascontiguousarray(em[sl]),
                "wkvT": wkvT,
                "wqT": wqT,
                "wpT": wpT,
                "biasb": biasb,
            }
        )
    return in_maps


class Runner:
    """Jitted SPMD executor with device-resident inputs for repeat timing."""

    def __init__(self):
        import jax
        from concourse.bass2jax import (
            _bass_exec_p,
            install_neuronx_cc_hook,
            partition_id_tensor,
        )
        from jax.experimental.shard_map import shard_map
        from jax.sharding import Mesh, PartitionSpec

        self.jax = jax
        nc = build()
        install_neuronx_cc_hook()
        pname = nc.partition_id_tensor.name if nc.partition_id_tensor else None
        in_names, out_names, out_avals = [], [], []
        for alloc in nc.m.functions[0].allocations:
            if not isinstance(alloc, mybir.MemoryLocationSet):
                continue
            name = alloc.memorylocations[0].name
            if alloc.kind == "ExternalInput":
                if name != pname:
                    in_names.append(name)
            elif alloc.kind == "ExternalOutput":
                out_names.append(name)
                out_avals.append(
                    jax.core.ShapedArray(
                        tuple(alloc.tensor_shape), mybir.dt.np(alloc.dtype)
                    )
                )
        self.in_names = list(in_names)
        self.out_names = out_names
        self.out_avals = out_avals
        n_params = len(in_names)
        all_names = in_names + out_names
        if pname is not None:
            all_names = all_names + [pname]
        donate = tuple(range(n_params, n_params + len(out_names)))

        def _body(*args):
            operands = list(args)
            if pname is not None:
                operands.append(partition_id_tensor())
            outs = _bass_exec_p.bind(
                *operands,
                out_avals=tuple(out_avals),
                in_names=tuple(all_names),
                out_names=tuple(out_names),
                lowering_input_output_aliases=(),
                sim_require_finite=True,
                sim_require_nnan=True,
                nc=nc,
            )
            return tuple(outs)

        devices = jax.devices()[:NCORES]
        self.mesh = Mesh(np.asarray(devices), ("core",))
        self.pspec = PartitionSpec("core")
        in_specs = (self.pspec,) * (n_params + len(out_names))
        out_specs = (self.pspec,) * len(out_names)
        self.fn = jax.jit(
            shard_map(
                _body,
                mesh=self.mesh,
                in_specs=in_specs,
                out_specs=out_specs,
                check_rep=False,
            ),
            donate_argnums=donate,
            keep_unused=True,
        )

    def put_inputs(self, in_maps):
        """Concat per-core inputs on axis 0 and move to devices (sharded)."""
        from jax.sharding import NamedSharding

        sh = NamedSharding(self.mesh, self.pspec)
        dev = []
        for name in self.in_names:
            cat = np.concatenate([m[name] for m in in_maps], axis=0)
            dev.append(self.jax.device_put(cat, sh))
        return dev

    def zeros(self):
        from jax.sharding import NamedSharding

        sh = NamedSharding(self.mesh, self.pspec)
        return [
            self.jax.device_put(
                np.zeros((NCORES * a.shape[0], *a.shape[1:]), a.dtype), sh
            )
            for a in self.out_avals
        ]

    def run(self, dev_inputs):
        outs = self.fn(*dev_inputs, *self.zeros())
        self.jax.block_until_ready(outs)
        return outs


def get_runner():
    global _runner
    if _runner is None:
        _runner = Runner()
    return _runner


def kernel(q, kv, key_mask, Wq, Wkv, Wproj, bproj):
    r = get_runner()
    in_maps = prep_inputs(q, kv, key_mask, Wq, Wkv, Wproj, bproj)
    dev = r.put_inputs(in_maps)
    outs = r.run(dev)
    out = np.asarray(outs[0]).reshape(NCORES, BL, QN, DIM).reshape(B, QN, DIM)
    return out.astype(np.float32)
